# revision 8
# baseline (speedup 1.0000x reference)
"""LLaMA causal self-attention, 8-way head-tensor-parallel Trainium2 Bass kernel.

v3: fp8(e4m3) DoubleRow matmuls with hi+lo residual splits where precision
requires it, PE-side causal masking, and fine-grained phase interleaving.

Sharding: each of 8 cores computes 4 query heads + its KV head-group
(Wq/Wk/Wv column-sharded), plus a row-shard of Wo producing a partial
(S, DIM) output; partials are summed on the host.

Engine plan (per core, CoreSim):
  - Projections: x and W split into fp8 hi+lo (W pre-scaled x64); 3-term
    DoubleRow matmuls (W8X8, dW8X8, W8dX8) -> ~bf16 accuracy at 0.75x
    bf16 cycle cost.
  - Scores: DR with lhsT slots (k8, dk8) x rhs (q8 broadcast) -> (k8+dk8)^T q8
    in one 256-cycle matmul per (t,j,head); q stays pure fp8.
  - Causal mask applied on PE: rank-128 DR matmul accumulates -240 into
    masked score elements => exp emits exact zeros; no DVE mask ops.
  - exp on ACT (the floor: ~73us), sliced to the causal column range.
  - AV: DR over j-tile pairs, lhsT (v8 pair) + second matmul (dv8 pair);
    softmax row-sums via a ones column in v8.
  - Out-projection: bf16 (fp8 fails the 2e-2 gate), K=128 x 2 ch-groups.
  - Normalization: DVE reciprocal off the psum row, rank-1 F32R broadcast
    matmul, Pool psum->sbuf copies, DVE multiplies.
"""

import numpy as np
import ml_dtypes  # noqa: F401

import concourse.bass as bass
import concourse.mybir as mybir
import concourse.tile as tile
from contextlib import nullcontext
from concourse import bacc
from concourse.bass import ts, ds
from concourse.bass_utils import run_bass_kernel_spmd
from concourse.masks import make_identity

F32 = mybir.dt.float32
F32R = mybir.dt.float32r
BF16 = mybir.dt.bfloat16
FP8 = mybir.dt.float8e4
DR = mybir.MatmulPerfMode.DoubleRow
EXP = mybir.ActivationFunctionType.Exp
COPY = mybir.ActivationFunctionType.Copy
MULT = mybir.AluOpType.mult
ADD = mybir.AluOpType.add

S = 2048
DIM = 2048
H = 32
KVH = 8
D = 64
NCORES = 8
HQ = H // NCORES          # 4 q heads per core
CQ = HQ * D               # 256 q cols per core
ST = 512
QT = 512
NKT = S // 128            # 16 key tiles
NDT = DIM // 128          # 16 contraction tiles
NST = S // ST
NQT = S // QT
WSCALE = 64.0
NEG = -240.0              # causal mask additive constant (exact in fp8)

SCORES_FP8 = True
AV_FP8 = True
DEBUG_DUMP = False


def _build(causal: bool, use_mask: bool):
    nc = bacc.Bacc("TRN2", target_bir_lowering=False, debug=False,
                   num_devices=NCORES, name="llama_attn")
    x8d = nc.dram_tensor("x8", [DIM, S], FP8, kind="ExternalInput")
    dx8d = nc.dram_tensor("dx8", [DIM, S], FP8, kind="ExternalInput")
    wq8d = nc.dram_tensor("wq8", [DIM, CQ], FP8, kind="ExternalInput")
    dwq8d = nc.dram_tensor("dwq8", [DIM, CQ], FP8, kind="ExternalInput")
    wkv8d = nc.dram_tensor("wkv8", [DIM, 128], FP8, kind="ExternalInput")
    dwkv8d = nc.dram_tensor("dwkv8", [DIM, 128], FP8, kind="ExternalInput")
    wod = nc.dram_tensor("wo", [CQ, DIM], BF16, kind="ExternalInput")
    bqd = nc.dram_tensor("bq", [CQ], F32, kind="ExternalInput")
    bkvd = nc.dram_tensor("bkv", [128], F32, kind="ExternalInput")
    ccd = nc.dram_tensor("cc", [128, S], BF16, kind="ExternalInput")
    ssd = nc.dram_tensor("ssgn", [128, S], BF16, kind="ExternalInput")
    ones8d = nc.dram_tensor("ones8", [128], FP8, kind="ExternalInput")
    onesfd = nc.dram_tensor("onesf", [64], F32R, kind="ExternalInput")
    if causal:
        tri8d = nc.dram_tensor("tri8", [128, QT], FP8, kind="ExternalInput")
        id8d = nc.dram_tensor("id8", [128, 2, 128], FP8, kind="ExternalInput")
    if use_mask:
        masktd = nc.dram_tensor("maskt", [S, S], BF16, kind="ExternalInput")
    partial = nc.dram_tensor("partial", [S, DIM], BF16, kind="ExternalOutput")
    if DEBUG_DUMP:
        PTDT0 = FP8 if AV_FP8 else BF16
        KTDT0 = FP8 if SCORES_FP8 else BF16
        qT_dd = nc.dram_tensor("qT_dd", [128, S], KTDT0, kind="ExternalOutput")
        kT_dd = nc.dram_tensor("kT_dd", [128, 2, S], KTDT0, kind="ExternalOutput")
        v8_dd = nc.dram_tensor("v8_dd", [128, NKT, 80], PTDT0, kind="ExternalOutput")
        dv8_dd = nc.dram_tensor("dv8_dd", [128, NKT, 80], FP8, kind="ExternalOutput")
        pt_dd = nc.dram_tensor("pt_dd", [128, 2, HQ, QT], PTDT0, kind="ExternalOutput")
        sv_dd = nc.dram_tensor("sv_dd", [65, QT], F32, kind="ExternalOutput")
        attn_dd = nc.dram_tensor("attn_dd", [128, QT], BF16, kind="ExternalOutput")
        kvr_dd = nc.dram_tensor("kvr_dd", [128, ST], BF16, kind="ExternalOutput")
        q0r_dd = nc.dram_tensor("q0r_dd", [128, ST], BF16, kind="ExternalOutput")

    PTDT = FP8 if AV_FP8 else BF16

    with tile.TileContext(nc) as tc:
        with tc.tile_pool(name="persist", bufs=1) as pp, \
             tc.tile_pool(name="xstream", bufs=2) as xp, \
             tc.tile_pool(name="scps", bufs=2, space="PSUM") as scp, \
             tc.tile_pool(name="avps", bufs=4, space="PSUM") as avp, \
             tc.tile_pool(name="ptp", bufs=6) as ptp, \
             tc.tile_pool(name="rope", bufs=3) as rp, \
             tc.tile_pool(name="nrm", bufs=8) as nrm, \
             tc.tile_pool(name="osb", bufs=8) as osb, \
             (tc.tile_pool(name="mskp", bufs=4) if use_mask else nullcontext()) as mskp:

            # ---------------- setup loads ----------------
            wq8 = pp.tile([128, NDT, CQ], FP8)
            dwq8 = pp.tile([128, NDT, CQ], FP8)
            wkv8 = pp.tile([128, NDT, 128], FP8)
            dwkv8 = pp.tile([128, NDT, 128], FP8)
            nc.sync.dma_start(
                wq8[:, ts(0, 8), :],
                wq8d.ap().rearrange("(a p) c -> p a c", p=128)[:, ts(0, 8), :])
            nc.scalar.dma_start(
                wq8[:, ts(1, 8), :],
                wq8d.ap().rearrange("(a p) c -> p a c", p=128)[:, ts(1, 8), :])
            nc.scalar.dma_start(
                wkv8[:], wkv8d.ap().rearrange("(a p) c -> p a c", p=128))
            nc.gpsimd.dma_start(
                dwq8[:], dwq8d.ap().rearrange("(a p) c -> p a c", p=128))
            nc.gpsimd.dma_start(
                dwkv8[:], dwkv8d.ap().rearrange("(a p) c -> p a c", p=128))
            bq_sb = pp.tile([128, 2], F32)
            nc.gpsimd.dma_start(bq_sb[:], bqd.ap().rearrange("(t p) -> p t", p=128))
            bkv_sb = pp.tile([128, 1], F32)
            nc.gpsimd.dma_start(bkv_sb[:], bkvd.ap()[:, None])
            cc_sb = pp.tile([128, S], BF16)
            ss_sb = pp.tile([128, S], BF16)
            wo_sb = pp.tile([128, 2, DIM], BF16)
            ones_pe = pp.tile([65, 64], F32R, name="ones_pe")
            nc.gpsimd.dma_start(ones_pe[64:65, :], onesfd.ap()[None, :])
            if causal:
                tri8 = pp.tile([128, QT], FP8)
                id8 = pp.tile([128, 2, 128], FP8)
                nc.gpsimd.dma_start(tri8[:], tri8d[:])
                nc.gpsimd.dma_start(id8[:], id8d[:])
            ident = pp.tile([64, 64], BF16)
            make_identity(nc, ident[:])

            # persistent activations
            if SCORES_FP8:
                qT = [pp.tile([128, S], FP8, name=f"qt{i}") for i in range(2)]
                kT = pp.tile([128, 2, S], FP8, name="kt")  # rows 64:128, slots (k8, dk8)
            else:
                qT = [pp.tile([128, S], BF16, name=f"qt{i}") for i in range(2)]
                kT = pp.tile([128, S], BF16, name="kt")    # rows 0:64 == 64:128
            v8 = pp.tile([128, NKT, 80], PTDT, name="v8")
            nc.gpsimd.dma_start(
                v8[:, :, 64:65],
                ones8d.ap()[:, None, None].to_broadcast((128, NKT, 1)))
            if AV_FP8:
                dv8 = pp.tile([128, NKT, 80], FP8, name="dv8")
                nc.gpsimd.memset(dv8[:, :, 64:65], 0.0)
            attn = [[pp.tile([128, QT], BF16, name=f"attn{c}_{t}")
                     for t in range(NQT)] for c in range(2)]

            # ---------------- emission helpers ----------------
            def proj_prefetch(st):
                ssl = ts(st, ST)
                xr8 = x8d.ap().rearrange("(a p) s -> p a s", p=128)
                dxr8 = dx8d.ap().rearrange("(a p) s -> p a s", p=128)
                xt = xp.tile([128, NDT, ST], FP8, tag="x8")
                dxt = xp.tile([128, NDT, ST], FP8, tag="dx8")
                if st == 0:
                    for xc in range(4):
                        nc.sync.dma_start(xt[:, ts(xc, 4), :],
                                          xr8[:, ts(xc, 4), ssl])
                else:
                    nc.sync.dma_start(xt[:], xr8[:, :, ssl])
                nc.gpsimd.dma_start(dxt[:], dxr8[:, :, ssl])
                if st == 0:
                    nc.scalar.dma_start(cc_sb[:], ccd[:])
                    nc.scalar.dma_start(ss_sb[:], ssd[:])
                if st == 1:
                    nc.sync.dma_start(
                        wo_sb[:], wod.ap().rearrange("(a p) e -> p a e", p=128))
                return xt, dxt

            def proj_mm(st, xt, dxt, kind, half, pq, lo, hi):
                """Emit matmuls [lo, hi) of the 24 (3 terms x 8 kpairs)."""
                if kind == "q":
                    wts = ((wq8, xt), (dwq8, xt), (wq8, dxt))
                    csl = ds(128 * half, 128)
                else:
                    wts = ((wkv8, xt), (dwkv8, xt), (wkv8, dxt))
                    csl = slice(None)
                if pq is None:
                    pq = scp.tile([128, 2, ST], F32, tag="sc",
                                  name=f"p{kind}{st}_{half}")
                for i in range(lo, hi):
                    wt, xtt = wts[i // 8]
                    kp = i % 8
                    nc.tensor.matmul(
                        pq[:, 0, :], wt[:, ts(kp, 2), csl], xtt[:, ts(kp, 2), :],
                        start=(i == 0), stop=(i == 23), perf_mode=DR)
                return pq

            def proj_part(st, xt, dxt, which):
                kind, half = (("q", 0), ("q", 1), ("kv", 0))[which]
                pq = proj_mm(st, xt, dxt, kind, half, None, 0, 24)
                proj_conv_rope(st, which, pq)

            def proj_conv_rope(st, which, pq):
                ssl = ts(st, ST)
                if which == 0:
                    q0r = rp.tile([128, ST], BF16, tag="q0r")
                    nc.vector.tensor_scalar(q0r[:], pq[:, 0, :], 1.0 / WSCALE,
                                            bq_sb[:, 0:1], MULT, ADD)
                    t1, sw, lo, hi = rope(q0r[:], [0, 64], ssl)
                    nc.vector.tensor_add(qT[0][:, ssl], t1[:], sw[:])
                elif which == 1:
                    q1r = rp.tile([128, ST], BF16, tag="q1r")
                    nc.vector.tensor_scalar(q1r[:], pq[:, 0, :], 1.0 / WSCALE,
                                            bq_sb[:, 1:2], MULT, ADD)
                    t1, sw, lo, hi = rope(q1r[:], [0, 64], ssl)
                    nc.vector.tensor_add(qT[1][:, ssl], t1[:], sw[:])
                else:
                    kvr = rp.tile([128, ST], BF16, tag="kvr")
                    nc.vector.tensor_scalar(kvr[:], pq[:, 0, :], 1.0 / WSCALE,
                                            bkv_sb[:, 0:1], MULT, ADD)
                    t1, sw, lo, hi = rope(kvr[:], [64], ssl)
                    if SCORES_FP8:
                        ktmp = rp.tile([128, ST], BF16, tag="ktmp")
                        nc.vector.tensor_add(ktmp[64:128, :], t1[64:128, :],
                                             sw[64:128, :])
                        nc.vector.tensor_copy(kT[64:128, 0, ssl], ktmp[64:128, :])
                        nc.vector.tensor_sub(kT[64:128, 1, ssl], ktmp[64:128, :],
                                             kT[64:128, 0, ssl])
                        nc.gpsimd.dma_start(kT[0:64, :, ssl], kT[64:128, :, ssl])
                    else:
                        nc.vector.tensor_add(kT[64:128, ssl], t1[64:128, :],
                                             sw[64:128, :])
                        nc.gpsimd.dma_start(kT[0:64, ssl], kT[64:128, ssl])
                    for vc in range(ST // 128):
                        jv = (st * ST) // 128 + vc
                        vt = scp.tile([128, 64], BF16, tag="sc", name=f"vt{jv}")
                        nc.tensor.transpose(vt[:], kvr[0:64, ts(vc, 128)], ident[:])
                        nc.vector.tensor_copy(v8[:, jv, 0:64], vt[:])
                        if AV_FP8:
                            nc.vector.tensor_sub(dv8[:, jv, 0:64], vt[:],
                                                 v8[:, jv, 0:64])

            def rope(raw, blocks, ssl):
                sw = rp.tile([128, ST], BF16, tag="sw")
                for b in blocks:
                    nc.sync.dma_start(sw[b:b + 32, :], raw[b + 32:b + 64, :])
                    nc.sync.dma_start(sw[b + 32:b + 64, :], raw[b:b + 32, :])
                lo, hi = blocks[0], blocks[-1] + 64
                t1 = rp.tile([128, ST], BF16, tag="t1")
                nc.vector.tensor_mul(t1[lo:hi, :], raw[lo:hi, :], cc_sb[lo:hi, ssl])
                nc.vector.tensor_mul(sw[lo:hi, :], sw[lo:hi, :], ss_sb[lo:hi, ssl])
                return t1, sw, lo, hi

            def _old_proj_finish(st, pq0, pq1, pkv):
                ssl = ts(st, ST)

                # psum -> sbuf conversions: x(1/WSCALE) + bias
                q0r = rp.tile([128, ST], BF16, tag="q0r")
                q1r = rp.tile([128, ST], BF16, tag="q1r")
                kvr = rp.tile([128, ST], BF16, tag="kvr")
                nc.vector.tensor_scalar(q0r[:], pq0[:, 0, :], 1.0 / WSCALE,
                                        bq_sb[:, 0:1], MULT, ADD)
                nc.vector.tensor_scalar(q1r[:], pq1[:, 0, :], 1.0 / WSCALE,
                                        bq_sb[:, 1:2], MULT, ADD)
                nc.vector.tensor_scalar(kvr[:], pkv[:, 0, :], 1.0 / WSCALE,
                                        bkv_sb[:, 0:1], MULT, ADD)
                if DEBUG_DUMP and st == 0:
                    nc.sync.dma_start(kvr_dd[:], kvr[:])
                    nc.sync.dma_start(q0r_dd[:], q0r[:])

                # RoPE
                def rope(raw, blocks):
                    sw = rp.tile([128, ST], BF16, tag="sw")
                    for b in blocks:
                        nc.sync.dma_start(sw[b:b + 32, :], raw[b + 32:b + 64, :])
                        nc.sync.dma_start(sw[b + 32:b + 64, :], raw[b:b + 32, :])
                    lo, hi = blocks[0], blocks[-1] + 64
                    t1 = rp.tile([128, ST], BF16, tag="t1")
                    nc.vector.tensor_mul(t1[lo:hi, :], raw[lo:hi, :], cc_sb[lo:hi, ssl])
                    nc.vector.tensor_mul(sw[lo:hi, :], sw[lo:hi, :], ss_sb[lo:hi, ssl])
                    return t1, sw, lo, hi

                t1, sw, lo, hi = rope(q0r[:], [0, 64])
                nc.vector.tensor_add(qT[0][:, ssl], t1[:], sw[:])
                t1, sw, lo, hi = rope(q1r[:], [0, 64])
                nc.vector.tensor_add(qT[1][:, ssl], t1[:], sw[:])
                t1, sw, lo, hi = rope(kvr[:], [64])
                if SCORES_FP8:
                    ktmp = rp.tile([128, ST], BF16, tag="ktmp")
                    nc.vector.tensor_add(ktmp[64:128, :], t1[64:128, :], sw[64:128, :])
                    nc.vector.tensor_copy(kT[64:128, 0, ssl], ktmp[64:128, :])
                    nc.vector.tensor_sub(kT[64:128, 1, ssl], ktmp[64:128, :],
                                         kT[64:128, 0, ssl])
                    nc.gpsimd.dma_start(kT[0:64, :, ssl], kT[64:128, :, ssl])
                else:
                    nc.vector.tensor_add(kT[64:128, ssl], t1[64:128, :],
                                         sw[64:128, :])
                    nc.gpsimd.dma_start(kT[0:64, ssl], kT[64:128, ssl])

                # v transpose (+ fp8 split)
                for vc in range(ST // 128):
                    j = (st * ST) // 128 + vc
                    vt = scp.tile([128, 64], BF16, tag="sc", name=f"vt{j}")
                    nc.tensor.transpose(vt[:], kvr[0:64, ts(vc, 128)], ident[:])
                    nc.vector.tensor_copy(v8[:, j, 0:64], vt[:])
                    if AV_FP8:
                        nc.vector.tensor_sub(dv8[:, j, 0:64], vt[:], v8[:, j, 0:64])

            def emit_outproj(t, sl, ep, idx):
                ot = osb.tile([128, 2, QT], BF16, tag="ot")
                engs = ("dve", "act", "dve", "dve", "dve", "dve", "act", "dve",
                        "dve", "dve", "dve", "act", "dve", "dve", "act", "dve")
                for i in range(2):
                    et = 2 * ep + i
                    pps = avp.tile([128, QT], F32, tag="av", name="op")
                    nc.tensor.matmul(pps[:], attn[0][t][:, ts(sl, 128)],
                                     wo_sb[:, 0, ts(et, 512)], start=True, stop=False)
                    nc.tensor.matmul(pps[:], attn[1][t][:, ts(sl, 128)],
                                     wo_sb[:, 1, ts(et, 512)], start=False, stop=True)
                    eng = engs[(2 * idx + i) % len(engs)]
                    if eng == "act":
                        nc.scalar.activation(ot[:, i, :], pps[:], COPY)
                    elif eng == "dve":
                        nc.vector.tensor_copy(ot[:, i, :], pps[:])
                    else:
                        nc.gpsimd.tensor_copy(ot[:, i, :], pps[:])
                nc.sync.dma_start(
                    partial[ts(4 * t + sl, 128), ds(1024 * ep, 1024)],
                    ot[:].rearrange("p a b -> p (a b)"))

            def finish_norm(t, svs, rcbs, hs=range(HQ)):
                for h in hs:
                    hp, hh = divmod(h, 2)
                    bc = scp.tile([128, 2, QT], F32, tag="sc", name="bc")
                    nc.tensor.matmul(bc[0:64, 0, :], ones_pe[64:65, 0:64],
                                     rcbs[h][64:65, :], start=True, stop=True)
                    if hh == 0:
                        nc.vector.tensor_mul(attn[hp][t][0:64, :],
                                             svs[h][0:64, :], bc[0:64, 0, :])
                    else:
                        tb = nrm.tile([64, QT], BF16, tag="tb")
                        nc.vector.tensor_mul(tb[:], svs[h][0:64, :],
                                             bc[0:64, 0, :])
                        nc.gpsimd.dma_start(attn[hp][t][64:128, :], tb[:])

            def score_exp(t, j, hp, ptt):
                jsl = ts(j, 128)
                d = j - 4 * t
                diag = causal and d >= 0
                off = 128 * d if diag else 0
                W = QT - off
                tsl0 = QT * t
                sc = scp.tile([128, 2, QT], F32, tag="sc", name="sc")
                for hh in range(2):
                    qsl = ds(tsl0 + off, W)
                    if SCORES_FP8:
                        rhs = qT[hp][ds(64 * hh, 64), None, qsl] \
                            .to_broadcast((64, 2, W))
                        nc.tensor.matmul(sc[:, hh, off:],
                                         kT[ds(64 * hh, 64), :, jsl],
                                         rhs, start=True, stop=not diag,
                                         perf_mode=DR)
                    else:
                        nc.tensor.matmul(sc[:, hh, off:],
                                         kT[ds(64 * hh, 64), jsl],
                                         qT[hp][ds(64 * hh, 64), qsl],
                                         start=True, stop=not diag)
                    if diag:
                        trhs = tri8[:, None, 0:W].to_broadcast((128, 2, W))
                        nc.tensor.matmul(sc[:, hh, off:], id8[:],
                                         trhs, start=False, stop=True,
                                         perf_mode=DR)
                nc.scalar.activation(
                    ptt[:, j % 2, ds(2 * hp, 2), off:],
                    sc[:, :, off:], EXP, scale=0.125)

            # ---------------- main fused loop ----------------
            xt0, dxt0 = proj_prefetch(0)
            for w in (0, 2, 1):
                proj_part(0, xt0, dxt0, w)
            hoist_ptt = None
            proj_state = None
            pending = []
            norm_prev = None
            op_idx = 0
            for t in range(NQT):
                tsl0 = QT * t
                n_k = 4 * (t + 1) if causal else NKT
                aps = [avp.tile([128, QT], F32, tag="av", name=f"av{t}_{h}")
                       for h in range(HQ)]
                if t < NQT - 1:
                    proj_state = {"x": proj_prefetch(t + 1)}
                for j in range(n_k):
                    jsl = ts(j, 128)
                    d = j - 4 * t
                    diag = causal and d >= 0
                    off = 128 * d if diag else 0
                    W = QT - off
                    if j == 2 and norm_prev is not None:
                        finish_norm(*norm_prev)
                        norm_prev = None
                    if proj_state is not None and j >= 1:
                        xt, dxt = proj_state["x"]
                        nslots = n_k - 1
                        done = proj_state.get("done", 0)
                        slot_i = j - 1
                        target = (72 * (slot_i + 1) + nslots - 1) // nslots \
                            if slot_i < nslots - 1 else 72
                        while done < target:
                            ph, lo = divmod(done, 24)
                            hi = min(24, lo + (target - done))
                            kind, half = (("q", 0), ("q", 1), ("kv", 0))[ph]
                            key = f"p{ph}"
                            proj_state[key] = proj_mm(
                                t + 1, xt, dxt, kind, half,
                                proj_state.get(key), lo, hi)
                            done += hi - lo
                            if done == (ph + 1) * 24:
                                # part complete: convert+rope now, freeing
                                # its psum slot before the next part allocs
                                proj_conv_rope(t + 1, ph, proj_state[key])
                        proj_state["done"] = done
                        if done == 72:
                            proj_state = None
                    for _ in range(2 if len(pending) > 6 else 1):
                        if j >= 2 and pending:
                            emit_outproj(*pending.pop(0), op_idx)
                            op_idx += 1
                    if use_mask:
                        mt = mskp.tile([128, QT], BF16, tag="mt")
                        nc.sync.dma_start(mt[:], masktd[jsl, ts(t, QT)])

                    if j % 2 == 0:
                        ptt = ptp.tile([128, 2, HQ, QT], PTDT, tag="pt")
                        if diag and d == 0 and causal:
                            nc.gpsimd.memset(ptt[:, 1, :, 0:128], 0.0)
                        if diag and d == 2 and causal:
                            nc.gpsimd.memset(ptt[:, 1, :, 256:384], 0.0)
                    for hp in (0, 1):
                        score_exp(t, j, hp, ptt)
                        if use_mask:
                            for c in range(2):
                                nc.vector.tensor_mul(
                                    ptt[:, j % 2, 2 * hp + c, :],
                                    ptt[:, j % 2, 2 * hp + c, :], mt[:])

                    if DEBUG_DUMP and t == 0 and j == 1:
                        nc.sync.dma_start(pt_dd[:], ptt[:])
                    if j % 2 == 1:
                        m = j // 2
                        poff = 256 if (causal and d == 3) else 0
                        for h in range(HQ):
                            if AV_FP8:
                                nc.tensor.matmul(
                                    aps[h][0:65, poff:], v8[:, ts(m, 2), 0:65],
                                    ptt[:, :, h, poff:], start=(m == 0), stop=False,
                                    perf_mode=DR, skip_group_check=True)
                                nc.tensor.matmul(
                                    aps[h][0:65, poff:], dv8[:, ts(m, 2), 0:65],
                                    ptt[:, :, h, poff:], start=False,
                                    stop=(j == n_k - 1),
                                    perf_mode=DR, skip_group_check=True)
                            else:
                                for jj in range(2):
                                    nc.tensor.matmul(
                                        aps[h][0:65, poff:], v8[:, 2 * m + jj, 0:65],
                                        ptt[:, jj, h, poff:],
                                        start=(m == 0 and jj == 0),
                                        stop=(j == n_k - 1 and jj == 1),
                                        skip_group_check=True)
                if t == NQT - 1:
                    while pending:
                        emit_outproj(*pending.pop(0), op_idx)
                        op_idx += 1
                # psum -> sbuf snapshot (frees aps), then sbuf reciprocal of
                # the Z row + F32R rounding copy for the bc broadcast matmul
                svs, rcbs = [], []
                for h in range(HQ):
                    sv = nrm.tile([65, QT], F32, tag="sv")
                    nc.scalar.activation(sv[0:65, :], aps[h][0:65, :], COPY)
                    svs.append(sv)
                    rc = nrm.tile([65, QT], F32, tag="rc")
                    nc.vector.reciprocal_approx_fast(rc[0:65, :], sv[0:65, :])
                    rcb = nrm.tile([65, QT], F32R, tag="rcb")
                    nc.scalar.activation(rcb[64:65, :], rc[64:65, :], COPY)
                    rcbs.append(rcb)
                if DEBUG_DUMP and t == 0:
                    nc.sync.dma_start(sv_dd[:], svs[0][:])
                    nc.sync.dma_start(qT_dd[:], qT[0][:])
                    nc.sync.dma_start(kT_dd[:], kT[:])
                    nc.sync.dma_start(v8_dd[:], v8[:])
                    if AV_FP8:
                        nc.sync.dma_start(dv8_dd[:], dv8[:])
                norm_prev = (t, svs, rcbs)
                pending.extend((t, sl, ep) for sl in range(4) for ep in range(2))
            if DEBUG_DUMP:
                nc.sync.dma_start(attn_dd[:], attn[0][0][:])
            # tail
            finish_norm(*norm_prev)
            while pending:
                emit_outproj(*pending.pop(0), op_idx)
                op_idx += 1

    nc.compile()
    return nc


_CACHE = {}
TRACE = False
LAST_EXEC_NS = None
LAST_RES = None


def _get(causal, use_mask):
    key = (causal, use_mask)
    if key not in _CACHE:
        _CACHE[key] = _build(causal, use_mask)
    return _CACHE[key]


def _perm_eo(w):
    cols = np.concatenate([np.arange(0, 64, 2), np.arange(1, 64, 2)])
    return w[..., cols]


def _bf(a):
    return np.ascontiguousarray(
        np.asarray(a, dtype=np.float32).astype(ml_dtypes.bfloat16))


def _f8(a):
    return np.ascontiguousarray(
        np.asarray(a, dtype=np.float32).astype(ml_dtypes.float8_e4m3))


def _split8(a, scale=1.0):
    a = np.asarray(a, dtype=np.float32) * scale
    hi = a.astype(ml_dtypes.float8_e4m3)
    lo = (a - hi.astype(np.float32)).astype(ml_dtypes.float8_e4m3)
    return np.ascontiguousarray(hi), np.ascontiguousarray(lo)


def kernel(**inputs):
    x = np.asarray(inputs["x"], dtype=np.float32)
    fc = np.asarray(inputs["freqs_cos"], dtype=np.float32)
    fs = np.asarray(inputs["freqs_sin"], dtype=np.float32)
    mask = np.asarray(inputs["mask"])
    Wq = np.asarray(inputs["Wq"], dtype=np.float32)
    bq = np.asarray(inputs["bq"], dtype=np.float32)
    Wk = np.asarray(inputs["Wk"], dtype=np.float32)
    bk = np.asarray(inputs["bk"], dtype=np.float32)
    Wv = np.asarray(inputs["Wv"], dtype=np.float32)
    bv = np.asarray(inputs["bv"], dtype=np.float32)
    Wo = np.asarray(inputs["Wo"], dtype=np.float32)
    bo = np.asarray(inputs["bo"], dtype=np.float32)

    m2 = mask.reshape(S, S)
    if (m2 == 1).all():
        causal, use_mask = False, False
    elif np.array_equal(m2 != 0, np.tril(np.ones((S, S), dtype=bool))):
        causal, use_mask = True, False
    else:
        causal, use_mask = False, True
    nc = _get(causal, use_mask)

    x8, dx8 = _split8(x[0].T)
    cosT = np.asarray(fc.T, dtype=np.float32)  # (32, S)
    sinT = np.asarray(fs.T, dtype=np.float32)
    cc = _bf(np.tile(cosT, (4, 1)))
    ssgn = _bf(np.concatenate([-sinT, sinT, -sinT, sinT], axis=0))
    kl = np.arange(128)[:, None]
    qq = np.arange(QT)[None, :]
    tri8 = _f8(np.where(qq < kl, NEG, 0.0))           # [128, 512]
    id8 = np.zeros((128, 2, 128), dtype=np.float32)
    id8[:, 0, :] = np.eye(128)
    id8 = _f8(id8)

    Wq_h = Wq.reshape(DIM, H, D)
    bq_h = bq.reshape(H, D)
    Wk_h = Wk.reshape(DIM, KVH, D)
    bk_h = bk.reshape(KVH, D)

    in_maps = []
    for c in range(NCORES):
        hs = slice(HQ * c, HQ * (c + 1))
        wq_c = _perm_eo(Wq_h[:, hs, :]).reshape(DIM, CQ)
        bq_c = _perm_eo(bq_h[hs, :]).reshape(CQ)
        wk_c = _perm_eo(Wk_h[:, c, :])
        bk_c = _perm_eo(bk_h[c, :])
        wv_c = Wv[:, 64 * c:64 * (c + 1)]
        bv_c = bv[64 * c:64 * (c + 1)]
        wkv_c = np.concatenate([wv_c, wk_c], axis=1)
        bkv_c = np.concatenate([bv_c, bk_c])
        wo_c = Wo[CQ * c:CQ * (c + 1), :]
        wq8, dwq8 = _split8(wq_c, WSCALE)
        wkv8, dwkv8 = _split8(wkv_c, WSCALE)
        im = {
            "x8": x8, "dx8": dx8,
            "wq8": wq8, "dwq8": dwq8, "wkv8": wkv8, "dwkv8": dwkv8,
            "wo": _bf(wo_c),
            "bq": np.ascontiguousarray(bq_c.astype(np.float32)),
            "bkv": np.ascontiguousarray(bkv_c.astype(np.float32)),
            "cc": cc, "ssgn": ssgn,
            "ones8": np.ones(128, dtype=ml_dtypes.float8_e4m3),
            "onesf": np.ones(64, dtype=np.float32),
        }
        if causal:
            im["tri8"] = tri8
            im["id8"] = id8
        if use_mask:
            im["maskt"] = _bf(m2.T)
        in_maps.append(im)

    global LAST_EXEC_NS, LAST_RES
    res = run_bass_kernel_spmd(nc, in_maps, core_ids=list(range(NCORES)),
                               trace=TRACE)
    LAST_EXEC_NS = res.exec_time_ns
    LAST_RES = res
    out = np.zeros((S, DIM), dtype=np.float32)
    for rr in res.results:
        out += np.asarray(rr["partial"], dtype=np.float32)
    out += bo
    return out.reshape(1, S, DIM)
